# revision 15
# baseline (speedup 1.0000x reference)
"""Trainium2 Bass kernel for nn_DeepLinearNet (64 chained 3x3 linear layers).

Math: reference computes h <- h @ W_l^T sequentially for l = 0..63, i.e.
    y = x @ (W_63 @ ... @ W_0)^T.
The 64-layer chain collapses (in float64, on host) to one 3x3 matrix A with
    y = x @ A,  A = (W_63 @ ... @ W_0)^T.
The device kernel is then a pure streaming op: read x (batch, 3), write
y = x @ A (batch, 3), with the 9 coefficients of A baked into the
instruction stream as immediates.

Sharding: pure data parallelism — batch split contiguously across 8 cores.

Per-core layout: rows are distributed so SBUF partition p of tile t holds
rows [(t*128 + p) * R_TILE, ...) — each partition's chunk is contiguous in
DRAM, so DMAs are full-bandwidth. Compute per output component j:
    t_j = x[:, 0] * A[0, j]                      (ScalarE activation-copy)
    t_j = x[:, 1] * A[1, j] + t_j                (VectorE scalar_tensor_tensor)
    y[:, j] = x[:, 2] * A[2, j] + t_j            (VectorE scalar_tensor_tensor)
using stride-3 access patterns on the interleaved (rows, 3) tiles.
"""

import numpy as np

import concourse.bacc as bacc
import concourse.bass as bass
import concourse.mybir as mybir
import concourse.tile as tile
from concourse.bass_utils import run_bass_kernel_spmd

N_CORES = 8
BATCH = 8388608
R_CORE = BATCH // N_CORES  # 1048576 rows per core
P = 128
R_PART = R_CORE // P  # 8192 rows per partition
R_TILE = 1024  # rows per partition per tile
N_TILES = R_PART // R_TILE  # 8
F32 = mybir.dt.float32


def _build_bass(
    scales: np.ndarray,
    r_tile: int = R_TILE,
    bufs_io: int = 3,
    bufs_tmp: int = 2,
    reps: int = 1,
    loop_reps: int = 0,
    out_on_act: bool = True,
    dma_mode: str = "compute",  # "compute" | "memcpy" | "split"
) -> bacc.Bacc:
    """Build the per-core Bass module. scales[c][j] = A[c, j] (f32).

    reps > 1 repeats the whole pass back-to-back (same data) — used only for
    steady-state hardware timing via wall-clock deltas.
    """
    a = [[float(scales[c, j]) for j in range(3)] for c in range(3)]
    if isinstance(r_tile, int):
        tile_sizes = [r_tile] * (R_PART // r_tile)
    else:
        tile_sizes = list(r_tile)
    assert sum(tile_sizes) == R_PART
    r_max = max(tile_sizes)
    nc = bacc.Bacc("TRN2", debug=False, num_devices=N_CORES)
    x = nc.dram_tensor("x", [R_CORE, 3], F32, kind="ExternalInput")
    y = nc.dram_tensor("y", [R_CORE, 3], F32, kind="ExternalOutput")
    xf = x.ap().rearrange("n c -> (n c)")
    yf = y.ap().rearrange("n c -> (n c)")

    def dram_view(flat, row0, r):
        # partition p gets rows [row0 + p*r, row0 + (p+1)*r) — contiguous
        return flat[3 * row0 : 3 * (row0 + P * r)].rearrange(
            "(p e) -> p e", p=P
        )

    with tile.TileContext(nc) as tc:
        with (
            tc.tile_pool(name="xin", bufs=bufs_io) as xin_pool,
            tc.tile_pool(name="yout", bufs=bufs_io) as yout_pool,
            tc.tile_pool(name="tmp", bufs=bufs_tmp) as tmp_pool,
        ):
            def one_pass():
                row0 = 0
                for ti, r in enumerate(tile_sizes):
                    xt = xin_pool.tile([P, 3 * r_max], F32, tag="xt")
                    if dma_mode == "split":
                        h = 3 * r // 2
                        nc.sync.dma_start(
                            xt[:, :h], dram_view(xf, row0, r)[:, :h]
                        )
                        nc.scalar.dma_start(
                            xt[:, h : 3 * r], dram_view(xf, row0, r)[:, h:]
                        )
                    else:
                        nc.sync.dma_start(xt[:, : 3 * r], dram_view(xf, row0, r))
                    if dma_mode == "memcpy":
                        nc.sync.dma_start(dram_view(yf, row0, r), xt[:, : 3 * r])
                        row0 += P * r
                        continue
                    yt = yout_pool.tile([P, 3 * r_max], F32, tag="yt")
                    x3 = xt[:, : 3 * r].rearrange("p (r c) -> p r c", c=3)
                    y3 = yt[:, : 3 * r].rearrange("p (r c) -> p r c", c=3)
                    for j in range(3):
                        tj = tmp_pool.tile([P, r_max], F32, tag=f"t{j}")
                        nc.scalar.mul(tj[:, :r], x3[:, :, 0], a[0][j])
                        nc.vector.scalar_tensor_tensor(
                            tj[:, :r], x3[:, :, 1], a[1][j], tj[:, :r],
                            mybir.AluOpType.mult, mybir.AluOpType.add,
                        )
                        nc.vector.scalar_tensor_tensor(
                            y3[:, :, j], x3[:, :, 2], a[2][j], tj[:, :r],
                            mybir.AluOpType.mult, mybir.AluOpType.add,
                        )
                    if dma_mode == "split":
                        h = 3 * r // 2
                        nc.scalar.dma_start(
                            dram_view(yf, row0, r)[:, :h], yt[:, :h]
                        )
                        nc.sync.dma_start(
                            dram_view(yf, row0, r)[:, h:], yt[:, h : 3 * r]
                        )
                    elif out_on_act:
                        nc.scalar.dma_start(dram_view(yf, row0, r), yt[:, : 3 * r])
                    else:
                        nc.sync.dma_start(dram_view(yf, row0, r), yt[:, : 3 * r])
                    row0 += P * r

            if loop_reps > 0:
                with tc.For_i(0, loop_reps, 1):
                    one_pass()
            else:
                for _rep in range(reps):
                    one_pass()
    nc.compile()
    return nc


_cache: dict[bytes, bacc.Bacc] = {}


def _get_bass(W: np.ndarray) -> bacc.Bacc:
    key = W.tobytes()
    if key not in _cache:
        M = np.eye(3, dtype=np.float64)
        for l in range(W.shape[0]):
            M = W[l].astype(np.float64) @ M
        A = M.T.astype(np.float32)  # y = x @ A
        _cache[key] = _build_bass(A)
    return _cache[key]


def kernel(x: np.ndarray, W: np.ndarray) -> np.ndarray:
    x = np.ascontiguousarray(np.asarray(x), dtype=np.float32)
    W = np.ascontiguousarray(np.asarray(W), dtype=np.float32)
    assert x.shape == (BATCH, 3) and W.shape[1:] == (3, 3)
    nc = _get_bass(W)
    in_maps = [
        {"x": x[i * R_CORE : (i + 1) * R_CORE]} for i in range(N_CORES)
    ]
    res = run_bass_kernel_spmd(nc, in_maps, core_ids=list(range(N_CORES)))
    return np.concatenate([r["y"] for r in res.results], axis=0)


# revision 20
# speedup vs baseline: 1.1799x; 1.1799x over previous
"""Trainium2 Bass kernel for nn_DeepLinearNet (64 chained 3x3 linear layers).

Math: reference computes h <- h @ W_l^T sequentially for l = 0..63, i.e.
    y = x @ (W_63 @ ... @ W_0)^T.
The 64-layer chain collapses (in float64, on host) to one 3x3 matrix A with
    y = x @ A,  A = (W_63 @ ... @ W_0)^T.
The device kernel is then a pure streaming op: read x (batch, 3), write
y = x @ A (batch, 3), with the 9 coefficients of A baked into the
instruction stream as immediates.

Sharding: pure data parallelism — batch split contiguously across 8 cores.

Per-core layout: rows are distributed so SBUF partition p of tile t holds
rows [(t*128 + p) * R_TILE, ...) — each partition's chunk is contiguous in
DRAM, so DMAs are full-bandwidth. Compute per output component j:
    t_j = x[:, 0] * A[0, j]                      (ScalarE activation-copy)
    t_j = x[:, 1] * A[1, j] + t_j                (VectorE scalar_tensor_tensor)
    y[:, j] = x[:, 2] * A[2, j] + t_j            (VectorE scalar_tensor_tensor)
using stride-3 access patterns on the interleaved (rows, 3) tiles.
Loads are issued on the SP HWDGE ring and stores on the ACT HWDGE ring
(measured ~5 us/pass faster than a single ring); VectorE/ScalarE busy time
(~59/~21 us) hides entirely under the ~80 us of DMA, so the kernel runs at
the per-core HBM streaming roofline (~300 GB/s measured incl. R/W mix).
"""

import numpy as np

import concourse.bacc as bacc
import concourse.mybir as mybir
import concourse.tile as tile
from concourse.bass_utils import run_bass_kernel_spmd

N_CORES = 8
BATCH = 8388608
R_CORE = BATCH // N_CORES  # 1048576 rows per core
P = 128
R_PART = R_CORE // P  # 8192 rows per partition
# Rows per partition per tile: tapered schedule — small first/last tiles
# shorten the DMA ramp-up and drain (HW-measured ~15 us faster per pass
# than uniform 1024-row tiles).
R_TILE = [512, 512, 1024, 1024, 1024, 1024, 1024, 1024, 512, 512]
F32 = mybir.dt.float32


def _build_bass(
    scales: np.ndarray,
    r_tile=R_TILE,
    bufs_io: int = 4,
    bufs_tmp: int = 2,
    reps: int = 1,
    loop_reps: int = 0,
    out_on_act: bool = True,
    dma_mode: str = "compute",  # "compute" | "memcpy" | "split"
) -> bacc.Bacc:
    """Build the per-core Bass module. scales[c][j] = A[c, j] (f32).

    reps > 1 repeats the whole pass back-to-back (same data) — used only for
    steady-state hardware timing via wall-clock deltas.
    """
    a = [[float(scales[c, j]) for j in range(3)] for c in range(3)]
    if isinstance(r_tile, int):
        tile_sizes = [r_tile] * (R_PART // r_tile)
    else:
        tile_sizes = list(r_tile)
    assert sum(tile_sizes) == R_PART
    r_max = max(tile_sizes)
    nc = bacc.Bacc("TRN2", debug=False, num_devices=N_CORES)
    x = nc.dram_tensor("x", [R_CORE, 3], F32, kind="ExternalInput")
    y = nc.dram_tensor("y", [R_CORE, 3], F32, kind="ExternalOutput")
    xf = x.ap().rearrange("n c -> (n c)")
    yf = y.ap().rearrange("n c -> (n c)")

    def dram_view(flat, row0, r):
        # partition p gets rows [row0 + p*r, row0 + (p+1)*r) — contiguous
        return flat[3 * row0 : 3 * (row0 + P * r)].rearrange(
            "(p e) -> p e", p=P
        )

    with tile.TileContext(nc) as tc:
        with (
            tc.tile_pool(name="xin", bufs=bufs_io) as xin_pool,
            tc.tile_pool(name="yout", bufs=bufs_io) as yout_pool,
            tc.tile_pool(name="tmp", bufs=bufs_tmp) as tmp_pool,
        ):
            def one_pass():
                row0 = 0
                for ti, r in enumerate(tile_sizes):
                    xt = xin_pool.tile([P, 3 * r_max], F32, tag="xt")
                    if dma_mode == "split":
                        h = 3 * r // 2
                        nc.sync.dma_start(
                            xt[:, :h], dram_view(xf, row0, r)[:, :h]
                        )
                        nc.scalar.dma_start(
                            xt[:, h : 3 * r], dram_view(xf, row0, r)[:, h:]
                        )
                    else:
                        nc.sync.dma_start(xt[:, : 3 * r], dram_view(xf, row0, r))
                    if dma_mode == "memcpy":
                        nc.sync.dma_start(dram_view(yf, row0, r), xt[:, : 3 * r])
                        row0 += P * r
                        continue
                    yt = yout_pool.tile([P, 3 * r_max], F32, tag="yt")
                    x3 = xt[:, : 3 * r].rearrange("p (r c) -> p r c", c=3)
                    y3 = yt[:, : 3 * r].rearrange("p (r c) -> p r c", c=3)
                    for j in range(3):
                        tj = tmp_pool.tile([P, r_max], F32, tag=f"t{j}")
                        nc.scalar.mul(tj[:, :r], x3[:, :, 0], a[0][j])
                        nc.vector.scalar_tensor_tensor(
                            tj[:, :r], x3[:, :, 1], a[1][j], tj[:, :r],
                            mybir.AluOpType.mult, mybir.AluOpType.add,
                        )
                        nc.vector.scalar_tensor_tensor(
                            y3[:, :, j], x3[:, :, 2], a[2][j], tj[:, :r],
                            mybir.AluOpType.mult, mybir.AluOpType.add,
                        )
                    if dma_mode == "split":
                        h = 3 * r // 2
                        nc.scalar.dma_start(
                            dram_view(yf, row0, r)[:, :h], yt[:, :h]
                        )
                        nc.sync.dma_start(
                            dram_view(yf, row0, r)[:, h:], yt[:, h : 3 * r]
                        )
                    elif out_on_act:
                        nc.scalar.dma_start(dram_view(yf, row0, r), yt[:, : 3 * r])
                    else:
                        nc.sync.dma_start(dram_view(yf, row0, r), yt[:, : 3 * r])
                    row0 += P * r

            if loop_reps > 0:
                with tc.For_i(0, loop_reps, 1):
                    one_pass()
            else:
                for _rep in range(reps):
                    one_pass()
    nc.compile()
    return nc


_cache: dict[bytes, bacc.Bacc] = {}


def _get_bass(W: np.ndarray) -> bacc.Bacc:
    key = W.tobytes()
    if key not in _cache:
        M = np.eye(3, dtype=np.float64)
        for l in range(W.shape[0]):
            M = W[l].astype(np.float64) @ M
        A = M.T.astype(np.float32)  # y = x @ A
        _cache[key] = _build_bass(A)
    return _cache[key]


def kernel(x: np.ndarray, W: np.ndarray) -> np.ndarray:
    x = np.ascontiguousarray(np.asarray(x), dtype=np.float32)
    W = np.ascontiguousarray(np.asarray(W), dtype=np.float32)
    assert x.shape == (BATCH, 3) and W.shape[1:] == (3, 3)
    nc = _get_bass(W)
    in_maps = [
        {"x": x[i * R_CORE : (i + 1) * R_CORE]} for i in range(N_CORES)
    ]
    res = run_bass_kernel_spmd(nc, in_maps, core_ids=list(range(N_CORES)))
    return np.concatenate([r["y"] for r in res.results], axis=0)
